# revision 16
# baseline (speedup 1.0000x reference)
"""Single-head attention (B=4, L=4096, EMB=312, HID=256) on 8 NeuronCores.

Sharding: data-parallel over batch (4) x key-parallel (2) = 8 cores. Each
core handles ALL 4096 queries against its half of the keys and returns the
UNNORMALIZED partial [sum_k p*v | sum_k p] rows; the host combines the two
halves as (o1+o2)/(s1+s2). Key-sharding (vs query-sharding) halves the
duplicated K/V projection work; only the Q projection is duplicated.

Per-core device algorithm (v2 — single-pass low precision, tolerance is
rel_err < 2e-2 so fp16/bf16 single-pass matmuls have ample headroom):
  - All matmuls are single-pass fp16 (projections, QK) or bf16 (PV): the
    cost model charges 1 cycle per output column regardless of contraction
    depth, so minimizing instruction count per output tile is what matters.
  - embT carries a ones-row at index EMB and W* carry the bias in that row,
    so projections fold the bias in. Wv has 2 extra columns: ones (gives the
    softmax row-sum through the P@V matmul) and zero padding (even N).
  - Each core's embT is column-permuted so its OWN key half occupies
    columns [0, KL): K/V projections read embT[:, :KL] directly (no
    separate embTk tensor) and the host un-permutes the output rows.
  - Scores are computed transposed: sT[kl, ql] = kT-chunk^T @ qT, so the
    exp() output is directly the stationary operand for the P@V matmul —
    no on-device transposes anywhere.
  - Mask is applied MULTIPLICATIVELY after exp: p = exp(s) * keep with
    keep = 1-mask in {0,1} (bf16, host-precomputed). Identical math to the
    additive -1e5 mask (exp of unmasked scores is finite in fp32), but the
    DVE multiply runs on 2-byte SBUF operands (fast path) instead of
    fp32 PSUM.
  - The raw partials (P@V columns + row-sum column) go back to the host,
    which normalizes after combining the key-halves.
"""
import os

import numpy as np
import ml_dtypes

import concourse.bacc as bacc
import concourse.tile as tile
from concourse import mybir, bass2jax
from concourse.bass_utils import run_bass_kernel_spmd

# Debug aid (opt-in): surface real compile errors from the PJRT compile
# hook, which the C++ bridge otherwise swallows.
if os.environ.get("BASS_KERNEL_DEBUG"):
    import functools as _ft
    import traceback as _tb
    _orig_hook = bass2jax.neuronx_cc_hook
    @_ft.wraps(_orig_hook)
    def _dbg_hook(*args, **kwargs):
        try:
            return _orig_hook(*args, **kwargs)
        except BaseException:
            _tb.print_exc()
            raise
    bass2jax.neuronx_cc_hook = _dbg_hook

EMB, HID, B, L = 312, 256, 4, 4096
NCORES = 8
P = 128
KL = L // 2            # key rows per core (key-parallel halves)
EPAD = 384             # emb dim padded to 3 partition chunks; row EMB is the ones-row
HV = HID + 2           # v columns: HID values | ones | zero pad (even N)
QT = 512               # ql tile width (PSUM bank = 512 fp32)
NKC = KL // P          # 16 kl chunks per core
NQT = L // QT          # 8 ql tiles per core (all queries)
NKT = KL // QT         # 4 l tiles for the k projection

F32 = mybir.dt.float32
F16 = mybir.dt.float16
BF16 = mybir.dt.bfloat16
FP8 = mybir.dt.float8e4
BF = ml_dtypes.bfloat16

_CACHE = {}


def _build(qk_mode):
    """qk_mode: "fp8dr" (fp8e4m3 hi/lo 3-term DoubleRow QK) or "fp16"
    (single-pass fp16 QK)."""
    qk_fp8 = qk_mode == "fp8dr"
    nc = bacc.Bacc(None)

    embT = nc.dram_tensor("embT", [EPAD, L], F16, kind="ExternalInput")
    wq = nc.dram_tensor("wq", [EPAD, HID], F16, kind="ExternalInput")
    wk = nc.dram_tensor("wk", [EPAD, HID], F16, kind="ExternalInput")
    wv = nc.dram_tensor("wv", [EPAD, HV], F16, kind="ExternalInput")
    keepT = nc.dram_tensor("keepT", [KL, L], BF16, kind="ExternalInput")
    out = nc.dram_tensor("out", [L, HID + 1], F32, kind="ExternalOutput")

    with tile.TileContext(nc) as tc:
        with (
            tc.tile_pool(name="big", bufs=1) as big,
            tc.tile_pool(name="wp", bufs=1) as wp,
            tc.tile_pool(name="keep", bufs=2) as kp,
            tc.tile_pool(name="pt", bufs=4) as ptp,
            tc.tile_pool(name="fin", bufs=2) as fin,
            tc.tile_pool(name="ps_st", bufs=4, space="PSUM") as ps_st,
            tc.tile_pool(name="ps_pv", bufs=1, space="PSUM") as ps_pv,
        ):
            # ---- load inputs (as [P, chunk, free] with the chunk index in
            # the free dim; partition line p reads rows {p, 128+p, 256+p}).
            # Weights ride the second HWDGE ring (ACT) so they don't
            # serialize ahead of the first embT blocks on the SP ring.
            def load_w(name, dram, ncol):
                t = wp.tile([P, 3, ncol], F16, name=name, tag=name)
                nc.scalar.dma_start(
                    out=t, in_=dram[:, :].rearrange("(c p) n -> p c n", p=P))
                return t

            embT_t = big.tile([P, 3, L], F16, name="embT")

            def load_emb(b0, blk):
                nc.sync.dma_start(
                    out=embT_t[:, :, b0:b0 + blk],
                    in_=embT[:, b0:b0 + blk].rearrange("(c p) n -> p c n", p=P),
                )

            # wk + first embT block first (they gate the first projection
            # matmul); remaining weights and blocks behind them.
            wk_t = load_w("wk", wk, HID)
            load_emb(0, QT)
            wv_t = load_w("wv", wv, HV)
            wq_t = load_w("wq", wq, HID)
            for b0 in range(QT, L, QT):
                load_emb(b0, QT)

            # ---- projections (single-pass fp16)
            # q/k in [h(part), hc, l(free)] layout; v in [kl(part), klc, h].
            # For the fp8 DoubleRow QK path, q/k are stored as an fp8e4m3
            # (hi, lo) pair: hi = round8(x), lo = round8(x - hi) gives ~8-bit
            # effective mantissa; the 3-term product hi@hi + lo@hi + hi@lo
            # drops only the ~2^-16-relative lo@lo term.
            kT = big.tile([P, 2, KL], F16, name="kT")
            qT = big.tile([P, 2, L], F16, name="qT")
            if qk_fp8:
                kT_h = big.tile([P, 2, KL], FP8, name="kT_h")
                kT_l = big.tile([P, 2, KL], FP8, name="kT_l")
                qT_h = big.tile([P, 2, L], FP8, name="qT_h")
                qT_l = big.tile([P, 2, L], FP8, name="qT_l")
            v_t = big.tile([P, NKC, HV], BF16, name="v")

            def emit_kq(which, hc, lt):
                ps = ps_st.tile([P, QT], F32, name="st", tag="st")
                w = wk_t if which == "k" else wq_t
                for c in range(3):
                    nc.tensor.matmul(
                        ps,
                        lhsT=w[:, c, hc * P:(hc + 1) * P],
                        rhs=embT_t[:, c, lt * QT:(lt + 1) * QT],
                        start=(c == 0), stop=(c == 2),
                    )
                # Single ACT copy to an fp16 staging tile frees the PSUM
                # bank quickly; the fp8 (hi, lo) split then runs on DVE from
                # SBUF (fast 2-byte path), off the PSUM critical ring.
                dst_t = kT if which == "k" else qT
                dst = (slice(None), hc, slice(lt * QT, (lt + 1) * QT))
                nc.scalar.copy(out=dst_t[dst], in_=ps)
                if qk_fp8:
                    dh, dl = (kT_h, kT_l) if which == "k" else (qT_h, qT_l)
                    nc.vector.tensor_copy(dh[dst], dst_t[dst])
                    nc.vector.tensor_sub(dl[dst], dst_t[dst], dh[dst])

            def emit_v(kc):
                ps = ps_st.tile([P, QT], F32, name="st", tag="st")
                for c in range(3):
                    nc.tensor.matmul(
                        ps[:, :HV],
                        lhsT=embT_t[:, c, kc * P:(kc + 1) * P],
                        rhs=wv_t[:, c, :],
                        start=(c == 0), stop=(c == 2),
                    )
                # Alternate the PSUM->SBUF copy between DVE and ACT so
                # neither copy engine gates the PE during projections.
                if kc % 2 == 0:
                    nc.vector.tensor_copy(v_t[:, kc, :], ps[:, :HV])
                else:
                    nc.scalar.copy(out=v_t[:, kc, :], in_=ps[:, :HV])

            # Interleave k/q tiles (PSUM->SBUF copies on ACT) with v tiles
            # (copies on DVE) so both copy engines run in parallel and
            # neither gates the PE.
            kq_tiles = [("k", hc, lt) for hc in range(2) for lt in range(NKT)]
            kq_tiles += [("q", hc, lt) for hc in range(2) for lt in range(NQT)]
            vi = 0
            for i, (which, hc, lt) in enumerate(kq_tiles):
                emit_kq(which, hc, lt)
                want_v = ((i + 1) * NKC) // len(kq_tiles)
                while vi < want_v:
                    emit_v(vi)
                    vi += 1
            while vi < NKC:
                emit_v(vi)
                vi += 1

            # ---- attention
            # Software-pipelined emission: chunk kc's P@V matmuls are emitted
            # AFTER chunk kc+1's QK matmuls, so the PE always has independent
            # work in program order while the ACT exp + DVE keep-multiply of
            # the current chunk are still in flight.
            # Query groups: 512-wide except the tail, which is split 256/256
            # so the final PSUM->SBUF->HBM drain after the last PV matmul is
            # short.
            qgroups = [(b0, QT) for b0 in range(0, L - QT, QT)]
            qgroups += [(L - QT, QT // 2), (L - QT // 2, QT // 2)]
            for gi, (q0, qw) in enumerate(qgroups):
                last = gi == len(qgroups) - 1
                nj = qw // P
                keep_t = kp.tile([P, NKC, qw], BF16, name="keep",
                                 tag=f"keep{qw}")
                nc.sync.dma_start(
                    out=keep_t,
                    in_=keepT[:, q0:q0 + qw].rearrange("(c p) n -> p c n", p=P),
                )
                pvs = [
                    ps_pv.tile([P, HV], F32, name=f"pv{j}", tag=f"pv{j}")
                    for j in range(nj)
                ]
                qsl = slice(q0, q0 + qw)
                pending_pv = None  # (kc, p-tile) awaiting PV emission

                def emit_pv(kc, ptile):
                    for j in range(nj):
                        nc.tensor.matmul(
                            pvs[j],
                            lhsT=ptile[:, j * P:(j + 1) * P],
                            rhs=v_t[:, kc, :],
                            start=(kc == 0), stop=(kc == NKC - 1),
                        )

                for kc in range(NKC):
                    st = ps_st.tile([P, QT], F32, name="st", tag="st")[:, :qw]
                    ksl = slice(kc * P, (kc + 1) * P)
                    if qk_fp8:
                        # DoubleRow: each matmul contracts both hid chunks
                        # (dim1 of the [P, 2, *] tiles) at 0.5 cycles/row.
                        terms = ((kT_h, qT_h), (kT_l, qT_h), (kT_h, qT_l))
                        for i, (kt_, qt_) in enumerate(terms):
                            nc.tensor.matmul(
                                st,
                                lhsT=kt_[:, :, ksl],
                                rhs=qt_[:, :, qsl],
                                start=(i == 0), stop=(i == 2),
                                perf_mode=mybir.MatmulPerfMode.DoubleRow,
                            )
                    else:
                        for hc in range(2):
                            nc.tensor.matmul(
                                st,
                                lhsT=kT[:, hc, ksl],
                                rhs=qT[:, hc, qsl],
                                start=(hc == 0), stop=(hc == 1),
                            )
                    if pending_pv is not None:
                        emit_pv(*pending_pv)
                    pe = ptp.tile([P, qw], BF16, name="pe", tag=f"pe{qw}")
                    nc.scalar.activation(
                        out=pe, in_=st, func=mybir.ActivationFunctionType.Exp)
                    pt = ptp.tile([P, qw], BF16, name="pt", tag=f"pt{qw}")
                    nc.vector.tensor_tensor(
                        out=pt, in0=pe, in1=keep_t[:, kc, :],
                        op=mybir.AluOpType.mult)
                    pending_pv = (kc, pt)
                emit_pv(*pending_pv)

                # Ship the unnormalized partial [sum p*v | sum p]; the host
                # divides after combining the two key-halves. One DMA per
                # group, except the last group which is split per-j (copies
                # alternating DVE/ACT, stores alternating HWDGE rings) so the
                # final drain isn't serialized on one engine.
                if not last:
                    ot = fin.tile([P, nj, HID + 1], F32, name="ot",
                                  tag=f"ot{qw}")
                    for j in range(nj):
                        nc.vector.tensor_copy(ot[:, j, :], pvs[j][:, :HID + 1])
                    nc.sync.dma_start(
                        out=out[qsl, :].rearrange("(j p) h -> p j h", p=P),
                        in_=ot,
                    )
                else:
                    for j in range(nj):
                        otj = fin.tile([P, HID + 1], F32, name="otj",
                                       tag=f"otj{j % 2}")
                        if j % 2 == 0:
                            nc.vector.tensor_copy(otj, pvs[j][:, :HID + 1])
                        else:
                            nc.scalar.copy(out=otj, in_=pvs[j][:, :HID + 1])
                        row0 = q0 + j * P
                        eng = nc.sync if j % 2 == 0 else nc.scalar
                        eng.dma_start(out=out[row0:row0 + P, :], in_=otj)
    nc.finalize()
    return nc


def _get_nc():
    qk_mode = os.environ.get("BASS_KERNEL_QK", "fp8dr")
    key = f"nc_{qk_mode}"
    if key not in _CACHE:
        _CACHE[key] = _build(qk_mode)
    return _CACHE[key]


def kernel(embedding, mask, Wq, bq, Wk, bk, Wv, bv):
    embedding = np.asarray(embedding, dtype=np.float32)
    mask = np.asarray(mask, dtype=np.float32)
    Wq = np.asarray(Wq, dtype=np.float32)
    Wk = np.asarray(Wk, dtype=np.float32)
    Wv = np.asarray(Wv, dtype=np.float32)
    bq = np.asarray(bq, dtype=np.float32)
    bk = np.asarray(bk, dtype=np.float32)
    bv = np.asarray(bv, dtype=np.float32)

    def pad_w(w, b, extra_one=False):
        wp = np.zeros((EPAD, HV if extra_one else HID), dtype=np.float32)
        wp[:EMB, :HID] = w
        wp[EMB, :HID] = b
        if extra_one:
            wp[EMB, HID] = 1.0
        return wp.astype(np.float16)

    wq_h = pad_w(Wq, bq)
    wk_h = pad_w(Wk, bk)
    wv_h = pad_w(Wv, bv, extra_one=True)

    in_maps = []
    for c in range(NCORES):
        b, half = divmod(c, 2)
        # Column-permute so this core's key half occupies columns [0, KL).
        eT = embedding[b].T  # [EMB, L]
        kp = (1.0 - mask[b].T)  # [L(k), L(q)]
        if half == 0:
            eTp = eT
            keep = kp[:KL, :]
        else:
            eTp = np.concatenate([eT[:, KL:], eT[:, :KL]], axis=1)
            keep = np.concatenate([kp[KL:, KL:], kp[KL:, :KL]], axis=1)
        embT = np.zeros((EPAD, L), dtype=np.float16)
        embT[:EMB] = eTp.astype(np.float16)
        embT[EMB] = 1.0
        in_maps.append({
            "embT": embT,
            "wq": wq_h, "wk": wk_h, "wv": wv_h,
            "keepT": np.ascontiguousarray(keep).astype(BF),
        })

    nc = _get_nc()
    trace = bool(int(os.environ.get("BASS_KERNEL_TRACE", "0")))
    res = run_bass_kernel_spmd(nc, in_maps, core_ids=list(range(NCORES)), trace=trace)
    _CACHE["last_results"] = res

    full = np.empty((B, L, HID), dtype=np.float32)
    for b in range(B):
        r0 = res.results[2 * b]["out"].astype(np.float64)
        r1 = res.results[2 * b + 1]["out"].astype(np.float64)
        # Core (b, 1) computed queries in permuted order [KL:L] + [0:KL];
        # un-permute its rows before combining.
        r1 = np.concatenate([r1[KL:], r1[:KL]], axis=0)
        num = r0[:, :HID] + r1[:, :HID]
        den = r0[:, HID:] + r1[:, HID:]
        full[b] = (num / den).astype(np.float32)
    return full
